# revision 19
# baseline (speedup 1.0000x reference)
"""Trainium2 Bass kernel for nn_AttentionOperation_1039382085848.

Per-core computation (tensor-parallel over heads, one head per NeuronCore):
  S = Q^T K  (logits, [L, M] per batch)
  BN2d(logits) over (N, L, M) per head, then softmax over M.
    Softmax is shift-invariant, so BN2d reduces to scaling logits by
    a = gamma_sim * rsqrt(var + eps). The stats are computed WITHOUT
    materializing S twice:
      sum(S)   = sum_b <qbar_b, kbar_b>         (qbar = row sums of Q)
      sum(S^2) = sum_b <Q_b Q_b^T, K_b K_b^T>   (Gram matrices, 64x64)
  S is computed transposed (S^T = K^T Q) so both matmuls use natural
  layouts; AV appends a ones-row to V^T so the softmax denominator falls
  out of the same PSUM accumulation.
  BN1d over (N, L) per channel + exact gelu fused into one activation.

Inputs are the FULL tensors; sharding over the 8 cores happens here.
"""

import numpy as np
import os
from contextlib import ExitStack

import concourse.bacc as bacc
import concourse.bass as bass
import concourse.mybir as mybir
import concourse.tile as tile
from concourse.bass_utils import run_bass_kernel_spmd
from concourse.masks import make_identity

N, H, D, L, M, C = 8, 8, 64, 1024, 1024, 64
EPS = 1e-5
NTOT = float(N * L * M)
f32 = mybir.dt.float32
f32r = mybir.dt.float32r
bf16 = mybir.dt.bfloat16
AF = mybir.ActivationFunctionType
X = mybir.AxisListType.X
GELU = AF.Identity if os.environ.get("KERNEL_SIM_GELU_ID") else AF.Gelu


def _body(ctx, nc, tc, q_ap, k_ap, v_ap, gs_ap, gv_ap, bv_ap, o_ap):
    const = ctx.enter_context(tc.tile_pool(name="const", bufs=1))
    work = ctx.enter_context(tc.tile_pool(name="work", bufs=2))
    psum = ctx.enter_context(tc.tile_pool(name="psum", bufs=2, space="PSUM"))
    dram = ctx.enter_context(tc.tile_pool(name="dram", bufs=1, space="DRAM"))

    # ---- constants ----
    ident = const.tile([128, 128], f32)
    make_identity(nc, ident)
    ones64 = const.tile([64, 1], f32)
    nc.vector.memset(ones64, 1.0)
    eps1 = const.tile([1, 1], f32)
    nc.vector.memset(eps1, EPS)
    eps64 = const.tile([64, 1], f32)
    nc.vector.memset(eps64, EPS)
    gs_t = const.tile([1, 1], f32)
    nc.sync.dma_start(out=gs_t, in_=gs_ap.rearrange("(a b) -> a b", b=1))
    gv_t = const.tile([64, 1], f32)
    nc.sync.dma_start(out=gv_t, in_=gv_ap.rearrange("(a b) -> a b", b=1))
    bv_t = const.tile([64, 1], f32)
    nc.sync.dma_start(out=bv_t, in_=bv_ap.rearrange("(a b) -> a b", b=1))

    # ---- persistent big tiles ----
    q_r = const.tile([D, N, L], f32r)      # Q (dram is f32r), 32KB/part
    k_r = const.tile([D, N, M], f32r)      # K (dram is f32r), 32KB/part
    vt = const.tile([128, N, 8, C + 1], f32r)  # V^T chunks + ones col
    rv = const.tile([C, N, L], f32)        # AV result, 32KB/part
    rvstats = const.tile([C, 2 * N, 6], f32)
    qkprod = const.tile([D, N], f32)       # per-b partials of sum(S)
    gprod = const.tile([D, N], f32)        # per-b partials of sum(S^2)

    # ones column of vt (f32r): memset can't write f32r, use ACT identity
    nc.scalar.activation(
        out=vt[:, :, :, C:C + 1],
        in_=ident[:, 0:N * 8].rearrange("p (a b c) -> p a b c", a=N, b=8, c=1),
        func=AF.Identity, scale=0.0, bias=1.0)

    # ================= Phase A: loads, V^T, Gram stats =================
    for b in range(N):
        nc.sync.dma_start(out=q_r[:, b, :], in_=q_ap[b])
        nc.sync.dma_start(out=k_r[:, b, :], in_=k_ap[b])
        v_st = work.tile([C, M], f32, tag="v_st", bufs=2)
        nc.sync.dma_start(out=v_st, in_=v_ap[b])

        qf = q_r[:, b, :].bitcast(f32)
        kf = k_r[:, b, :].bitcast(f32)
        qbar = work.tile([D, 1], f32, tag="qbar", bufs=2)
        nc.vector.reduce_sum(out=qbar, in_=qf, axis=X)
        kbar = work.tile([D, 1], f32, tag="kbar", bufs=2)
        nc.vector.reduce_sum(out=kbar, in_=kf, axis=X)
        nc.vector.tensor_mul(out=qkprod[:, b:b + 1], in0=qbar, in1=kbar)

        # transposes: pack 8 chunks of a tensor into one PSUM bank
        tpq = psum.tile([128, 8, D], f32, tag="tp", bufs=2)
        tpk = psum.tile([128, 8, D], f32, tag="tp", bufs=2)
        tpv = psum.tile([128, 8, D], f32, tag="tp", bufs=2)
        for ch in range(8):
            sl = slice(ch * 128, (ch + 1) * 128)
            nc.tensor.transpose(tpq[:, ch, :], qf[:, sl], ident[:D, :D])
            nc.tensor.transpose(tpk[:, ch, :], kf[:, sl], ident[:D, :D])
            nc.tensor.transpose(tpv[:, ch, :], v_st[:, sl], ident[:D, :D])
        qt = work.tile([128, 8, D], bf16, tag="qt", bufs=2)
        nc.vector.tensor_copy(out=qt, in_=tpq)
        kt = work.tile([128, 8, D], bf16, tag="kt", bufs=2)
        nc.vector.tensor_copy(out=kt, in_=tpk)
        nc.vector.tensor_copy(out=vt[:, b, :, 0:C], in_=tpv)

        gq = psum.tile([D, D], f32, tag="gav", bufs=2)
        gk = psum.tile([D, D], f32, tag="gav", bufs=2)
        for ch in range(8):
            nc.tensor.matmul(gq, lhsT=qt[:, ch, :], rhs=qt[:, ch, :],
                             start=(ch == 0), stop=(ch == 7))
            nc.tensor.matmul(gk, lhsT=kt[:, ch, :], rhs=kt[:, ch, :],
                             start=(ch == 0), stop=(ch == 7))
        gqs = work.tile([D, D], f32, tag="gqs", bufs=2)
        nc.vector.tensor_copy(out=gqs, in_=gq)
        gsc = work.tile([D, D], f32, tag="gsc", bufs=2)
        nc.vector.tensor_mul(out=gsc, in0=gqs, in1=gk)
        nc.vector.reduce_sum(out=gprod[:, b:b + 1], in_=gsc, axis=X)

    # ---- finalize BN2d scale a = gamma_sim * rsqrt(var + eps) ----
    spart = const.tile([64, 2], f32)
    nc.vector.reduce_sum(out=spart[:, 0:1], in_=qkprod, axis=X)
    nc.vector.reduce_sum(out=spart[:, 1:2], in_=gprod, axis=X)
    ssp = psum.tile([1, 2], f32, tag="tp", bufs=2)
    nc.tensor.matmul(ssp, lhsT=ones64, rhs=spart, start=True, stop=True)
    sc = const.tile([1, 8], f32)
    nc.scalar.copy(out=sc[:, 0:2], in_=ssp)
    nc.scalar.mul(out=sc[:, 2:3], in_=sc[:, 0:1], mul=1.0 / NTOT)   # mean
    nc.scalar.activation(out=sc[:, 3:4], in_=sc[:, 2:3], func=AF.Square)
    nc.scalar.mul(out=sc[:, 4:5], in_=sc[:, 1:2], mul=1.0 / NTOT)   # E[s^2]
    nc.vector.tensor_sub(out=sc[:, 5:6], in0=sc[:, 4:5], in1=sc[:, 3:4])
    nc.scalar.activation(out=sc[:, 6:7], in_=sc[:, 5:6], func=AF.Sqrt, bias=eps1)
    nc.vector.reciprocal(out=sc[:, 7:8], in_=sc[:, 6:7])
    a1 = const.tile([1, 1], f32)
    nc.vector.tensor_mul(out=a1, in0=sc[:, 7:8], in1=gs_t)
    a_b = const.tile([128, 1], f32)
    nc.gpsimd.partition_broadcast(a_b, a1)


    # ================= Phase B/C: S^T, exp, AV =================
    for b in range(N):
        w = work.tile([128, 8, M], f32r, tag="w", bufs=1)
        for mc in range(8):
            msl = slice(mc * 128, (mc + 1) * 128)
            sp = psum.tile([128, M], f32, tag="s", bufs=2)
            for lh in range(2):
                lsl = slice(lh * 512, (lh + 1) * 512)
                nc.tensor.matmul(sp[:, lsl], lhsT=k_r[:, b, msl],
                                 rhs=q_r[:, b, lsl], start=True, stop=True)
            nc.scalar.activation(out=w[:, mc, :], in_=sp, func=AF.Exp,
                                 scale=a_b)
        av0 = psum.tile([C + 1, 512], f32, tag="gav", bufs=2)
        av1 = psum.tile([C + 1, 512], f32, tag="gav", bufs=2)
        avs = [av0, av1]
        for mc in range(8):
            for lh in range(2):
                lsl = slice(lh * 512, (lh + 1) * 512)
                nc.tensor.matmul(avs[lh], lhsT=vt[:, b, mc, :],
                                 rhs=w[:, mc, lsl],
                                 start=(mc == 0), stop=(mc == 7))
        for lh in range(2):
            av = avs[lh]
            lsl = slice(lh * 512, (lh + 1) * 512)
            rden = work.tile([1, 512], f32, tag="rden", bufs=2)
            nc.vector.reciprocal(out=rden, in_=av[C:C + 1, :])
            rdb = work.tile([C, 512], f32, tag="rdb", bufs=2)
            nc.gpsimd.partition_broadcast(rdb, rden)
            nc.vector.tensor_mul(out=rv[:, b, lsl], in0=av[0:C, :], in1=rdb)
            nc.vector.bn_stats(out=rvstats[:, 2 * b + lh, :], in_=rv[:, b, lsl])

    # ================= Phase D: BN1d + gelu =================
    mv = const.tile([C, 2], f32)
    nc.vector.bn_aggr(out=mv, in_=rvstats)
    stdc = const.tile([C, 1], f32)
    nc.scalar.activation(out=stdc, in_=mv[:, 1:2], func=AF.Sqrt, bias=eps64)
    rstd = const.tile([C, 1], f32)
    nc.vector.reciprocal(out=rstd, in_=stdc)
    scale_c = const.tile([C, 1], f32)
    nc.vector.tensor_mul(out=scale_c, in0=rstd, in1=gv_t)
    tmpm = const.tile([C, 1], f32)
    nc.vector.tensor_mul(out=tmpm, in0=mv[:, 0:1], in1=scale_c)
    shift_c = const.tile([C, 1], f32)
    nc.vector.tensor_sub(out=shift_c, in0=bv_t, in1=tmpm)
    for b2 in range(N // 2):
        ot = work.tile([C, 2, L], f32, tag="ot", bufs=3)
        nc.scalar.activation(out=ot, in_=rv[:, 2 * b2:2 * b2 + 2, :], func=GELU,
                             scale=scale_c, bias=shift_c)
        nc.sync.dma_start(out=o_ap[2 * b2:2 * b2 + 2].rearrange("b c l -> c b l"),
                          in_=ot)


_NC_CACHE = None


def _build():
    global _NC_CACHE
    if _NC_CACHE is not None:
        return _NC_CACHE
    nc = bacc.Bacc("TRN2", target_bir_lowering=False, debug=False, num_devices=8)
    # q/k declared f32r so the fp32r matmuls read them directly from DMA
    # (numpy float32 maps to the same 4-byte layout; the PE rounds on read)
    q_d = nc.dram_tensor("q", [N, D, L], f32r, kind="ExternalInput")
    k_d = nc.dram_tensor("k", [N, D, M], f32r, kind="ExternalInput")
    v_d = nc.dram_tensor("v", [N, C, M], f32, kind="ExternalInput")
    gs_d = nc.dram_tensor("g_sim", [1], f32, kind="ExternalInput")
    gv_d = nc.dram_tensor("g_v", [C], f32, kind="ExternalInput")
    bv_d = nc.dram_tensor("b_v", [C], f32, kind="ExternalInput")
    o_d = nc.dram_tensor("out", [N, C, L], f32, kind="ExternalOutput")
    reps = int(os.environ.get("KERNEL_REPS", "1"))
    with tile.TileContext(nc) as tc:
        for _ in range(reps):
            with ExitStack() as ctx:
                _body(ctx, nc, tc, q_d.ap(), k_d.ap(), v_d.ap(),
                      gs_d.ap(), gv_d.ap(), bv_d.ap(), o_d.ap())
    nc.compile()
    _NC_CACHE = nc
    return nc


LAST_RESULTS = None
LAST_IN_MAPS = None


def kernel(query, key, value, gamma_sim, beta_sim, gamma_v, beta_v):
    global LAST_RESULTS, LAST_IN_MAPS
    query = np.ascontiguousarray(np.asarray(query, dtype=np.float32))
    key = np.ascontiguousarray(np.asarray(key, dtype=np.float32))
    value = np.ascontiguousarray(np.asarray(value, dtype=np.float32))
    gamma_sim = np.asarray(gamma_sim, dtype=np.float32)
    gamma_v = np.asarray(gamma_v, dtype=np.float32).reshape(H, C)
    beta_v = np.asarray(beta_v, dtype=np.float32).reshape(H, C)

    nc = _build()
    in_maps = []
    for h in range(H):
        in_maps.append({
            "q": np.ascontiguousarray(query[:, h]),
            "k": np.ascontiguousarray(key[:, h]),
            "v": np.ascontiguousarray(value[:, h]),
            "g_sim": np.ascontiguousarray(gamma_sim[h:h + 1]),
            "g_v": np.ascontiguousarray(gamma_v[h]),
            "b_v": np.ascontiguousarray(beta_v[h]),
        })
    LAST_IN_MAPS = in_maps
    res = run_bass_kernel_spmd(nc, in_maps, core_ids=list(range(8)))
    LAST_RESULTS = res
    out = np.empty((N, H * C, L), np.float32)
    for h in range(H):
        out[:, h * C:(h + 1) * C, :] = res.results[h]["out"]
    return out
